# revision 17
# baseline (speedup 1.0000x reference)
"""BitLinear-1.58 Trainium2 kernel (8-core SPMD), v8.

out = (clip(round(x * s), -128, 127) @ w.T) / s / weight_scale + bias,
s = 127 / clip(rowmax|x|, 1e-5),  w in {0,1} (int32), x [4096, 8192] f32.

v8 strategy (vs v7 = 4x2 token/outfeature split, int32 w streamed +
cast + XBAR-transposed on device, 1.63 ms):
  * Token-parallel 8-way: each core owns 512 tokens x full 8192 out
    features. Weight is replicated.
  * Weight is repacked ON HOST (static data, offline-transformable):
    int32 -> bf16 (exact for {0,1}) and pre-transposed into slab layout
    wtb[p=128, gi=1024, n=512] where gi = nt*8 + g covers ko = gi*8
    (k = ko*128 + p), n = nt*512 + nl. Every GEMM slab group is then a
    single 1 MB DMA, contiguous 8 KB per partition -> ~full DMA rate,
    zero on-device weight compute (v7 burned ~220us of ACT + 64 MB of
    XBAR on the weight path).
  * Phase X (activation quant) only covers 4 token tiles now (~55us
    serial head instead of ~230us), then the GEMM runs TensorE-bound:
    4096 matmuls [128k,128t]x[128k,512n] ~ 213ns each ~ 873us.
  * GEMM: nt-outer (16 n-tiles of 512), g-middle (8 slab groups of 8 ko),
    t-inner (4 token tiles); psum per (nt,t) accumulates all 64 ko.
    Slabs prefetched 3 groups ahead on the sync queue; drains (DVE scale
    by per-token d + bias add -> bf16 staging) land 24 matmuls before
    their psum is needed again; stores on gpsimd.

Carried-over v7 lessons: magic-number RNE round; approximate s (127 *
recip(m)) with exact output scale d = m/127/ws; bf16 output staging
(host widens); 2+2 sync/gpsimd queue split for x quarter loads; ring
depths sized so DMA-completion latency (~5-10us) never paces a pool.

Exactness: x_q ints in [-127,127] and w {0,1} are exact in bf16; every
partial sum < 2^24 so fp32 PSUM accumulation is exact. clip never binds
since |x*s| <= 127.
"""
import sys

sys.path.insert(0, "/opt/trn_rl_repo")

from contextlib import ExitStack

import numpy as np
import ml_dtypes

import concourse.bass as bass
import concourse.tile as tile
from concourse import bacc, mybir
from concourse.bass import ts
from concourse.bass_utils import run_bass_kernel_spmd

TOKENS, IN_F, OUT_F = 4096, 8192, 8192
A_SPLIT = 8                  # token blocks -> 8 cores
T_LOC = TOKENS // A_SPLIT    # 512
P = 128
KO = IN_F // P               # 64 k-tiles of 128
TT = T_LOC // P              # 4 token tiles
NT = OUT_F // 512            # 16 n-tiles of 512
KG = 8                       # ko per slab group
NG = KO // KG                # 8 slab groups per nt
GI_N = NT * NG               # 128 slab groups total
XHW = 2048                   # x chunk width in k
XHN = IN_F // XHW            # 4 quarters
MAGIC = float(np.float32(1.5 * 2 ** 23))

_CACHE = {}


def _build():
    if "nc" in _CACHE:
        return _CACHE["nc"]

    nc = bacc.Bacc("TRN2", target_bir_lowering=False, debug=False, num_devices=8)
    f32, bf16 = mybir.dt.float32, mybir.dt.bfloat16
    A = mybir.AluOpType

    xb = nc.dram_tensor("xb", [T_LOC, IN_F], f32, kind="ExternalInput").ap()
    wtb = nc.dram_tensor("wtb", [P, GI_N * KG, 512], bf16,
                         kind="ExternalInput").ap()
    bb = nc.dram_tensor("bb", [OUT_F], f32, kind="ExternalInput").ap()
    ws = nc.dram_tensor("ws", [1], f32, kind="ExternalInput").ap()
    ob = nc.dram_tensor("ob", [T_LOC, OUT_F], bf16, kind="ExternalOutput").ap()

    with tile.TileContext(nc) as tc:
        with ExitStack() as ctx:
            small = ctx.enter_context(tc.tile_pool(name="small", bufs=1))
            sp2 = ctx.enter_context(tc.tile_pool(name="sp2", bufs=2))
            xqp = ctx.enter_context(tc.tile_pool(name="xq", bufs=1))
            pp = ctx.enter_context(tc.tile_pool(name="psum", bufs=8, space="PSUM"))
            slp = ctx.enter_context(tc.tile_pool(name="slab", bufs=6))
            op = ctx.enter_context(tc.tile_pool(name="outp", bufs=8))
            bbp = ctx.enter_context(tc.tile_pool(name="bbc", bufs=2))

            xq = xqp.tile([P, KO, T_LOC], bf16)   # resident 8 MB xq cache

            # ---- prologue (DMA-only; no gpsimd DSP ops) ----
            rws_b = small.tile([P, 1], f32)
            nc.gpsimd.dma_start(rws_b[:], ws[None, :].broadcast_to((P, 1)))
            nc.vector.reciprocal(rws_b[:], rws_b[:])

            d_all = small.tile([P, TT], f32)      # per-token out scale 1/s/ws
            m_all = small.tile([P, TT], f32)

            # ---- ring plan (each proven in an earlier rev):
            #   sync:   8 x quarter-loads + 16 phase-X XBARs (head), then
            #           all steady-state slabs (147 GB/s, v8-clean)
            #   gpsimd: 8 x quarter-loads, bias broadcasts, output stores
            #   scalar: ONLY the 4-group slab lead during the head (a lone
            #           ring sustains just ~170 GB/s -- v10's chronic-lag
            #           lesson -- but an idle-head 4 MB lead is fine)
            # v8 lost ~45us at GEMM start because lead slabs queued behind
            # the head's XBARs on sync; v9/v11 lost more because slab
            # bursts stole HBM from the x loads. ----
            slabs = {}

            def emit_slab(gi, eng=None):
                slab = slp.tile([P, KG, 512], bf16, tag="slab",
                                name=f"slab_{gi}")
                (eng or nc.sync).dma_start(slab[:], wtb[:, ts(gi, KG), :])
                slabs[gi] = slab

            b_bcs = {}

            def emit_bias(nt):
                b_bc = bbp.tile([P, 512], f32, tag="bbc", name=f"bbc_{nt}")
                nc.gpsimd.dma_start(
                    b_bc[:], bb[None, ts(nt, 512)].broadcast_to((P, 512)))
                b_bcs[nt] = b_bc

            for gi in range(4):
                emit_slab(gi, eng=nc.scalar)
            emit_bias(0)

            # ---- Phase X: x -> s -> quantize -> XBAR into xq cache.
            # Loads are emitted ONE TILE AHEAD of the compute/XBAR ops:
            # engine queues are strict FIFO, so emitting tile t+1's loads
            # after tile t's XBARs (v8..v12) serialized phase X at
            # ~35us/tile -- the engine could not even ISSUE the next loads
            # until the previous tile's transposes completed. ----
            with tc.tile_pool(name="phx", bufs=8) as phx, \
                 tc.tile_pool(name="phxq", bufs=3) as phq:
                tile_q = {}

                def emit_loads(tt):
                    quarters = []
                    for q in range(XHN):
                        xh = phx.tile([P, XHW], f32, tag="xh")
                        # 2+2 queue split: one ring only sustains ~170 GB/s,
                        # two together reach the ~358 GB/s HBM limit
                        eng = nc.sync if q % 2 == 0 else nc.gpsimd
                        eng.dma_start(xh[:], xb[ts(tt, P), ts(q, XHW)])
                        quarters.append(xh)
                    tile_q[tt] = quarters

                emit_loads(0)
                for tt in range(TT):
                    if tt + 1 < TT:
                        emit_loads(tt + 1)
                    m4 = sp2.tile([P, XHN], f32, tag="m4")
                    quarters = tile_q.pop(tt)
                    for q in range(XHN):
                        nc.vector.tensor_reduce(
                            m4[:, q : q + 1], quarters[q][:],
                            mybir.AxisListType.X,
                            A.max, apply_absolute_value=True)
                    m_col = m_all[:, tt : tt + 1]
                    nc.vector.tensor_reduce(m_col, m4[:],
                                            mybir.AxisListType.X, A.max)
                    nc.vector.tensor_scalar_max(m_col, m_col, 1e-5)
                    s_t = sp2.tile([P, 1], f32, tag="s_t")
                    nc.vector.reciprocal(s_t[:], m_col)
                    nc.vector.tensor_scalar_mul(s_t[:], s_t[:], 127.0)
                    # d = m / 127 / ws  (exact chain, independent of s)
                    nc.vector.tensor_scalar(d_all[:, tt : tt + 1], m_col,
                                            rws_b[:, 0:1], 1.0 / 127.0,
                                            A.mult, A.mult)
                    for xh in quarters:
                        # ACT: xh = x*s + MAGIC in place (f32, exact int part)
                        nc.scalar.activation(xh[:], xh[:],
                                             mybir.ActivationFunctionType.Copy,
                                             bias=MAGIC, scale=s_t[:, 0:1])
                    for q, xh in enumerate(quarters):
                        # -MAGIC -> bf16 staging; split across DVE and ACT
                        xqh = phq.tile([P, XHW], bf16, tag="xqh")
                        if q % 2 == 0:
                            nc.vector.tensor_scalar_sub(xqh[:], xh[:], MAGIC)
                        else:
                            nc.scalar.activation(
                                xqh[:], xh[:],
                                mybir.ActivationFunctionType.Copy, bias=-MAGIC)
                        nc.sync.dma_start_transpose(
                            xq[:, ts(q, KO // XHN), ts(tt, P)], xqh[:])

            # ---- GEMM: g-outer / t-inner (slab consumption a gentle
            # 1 MB / 6.9us); psum per (nt,t) accumulates all 64 ko ----
            for nt in range(NT):
                psums = [pp.tile([P, 512], f32, tag="acc",
                                 name=f"ps_{nt}_{t}") for t in range(TT)]
                if nt + 1 < NT:
                    emit_bias(nt + 1)
                b_bc = b_bcs.pop(nt)
                for g in range(NG):
                    gi = nt * NG + g
                    if gi + 4 < GI_N:
                        emit_slab(gi + 4)
                    slab = slabs.pop(gi)
                    last = g == NG - 1
                    for t in range(TT):
                        for kol in range(KG):
                            ko = g * KG + kol
                            nc.tensor.matmul(
                                psums[t][:], xq[:, ko, ts(t, P)],
                                slab[:, kol, :],
                                start=(ko == 0), stop=(ko == KO - 1))
                        if last:
                            # drain right behind this tile's final matmul on
                            # DVE; bf16 staging (<= 2 final roundings)
                            o_sb = op.tile([P, 512], bf16, tag="osb",
                                           name=f"osb_{nt}_{t}")
                            nc.vector.tensor_scalar(o_sb[:], psums[t][:],
                                                    d_all[:, t : t + 1], None,
                                                    A.mult)
                            nc.vector.tensor_tensor(o_sb[:], o_sb[:], b_bc[:],
                                                    A.add)
                            nc.gpsimd.dma_start(ob[ts(t, P), ts(nt, 512)],
                                                o_sb[:])

    nc.compile()
    _CACHE["nc"] = nc
    return nc


def _pack_weight(weight: np.ndarray) -> np.ndarray:
    """int32 [OUT_F, IN_F] {0,1} -> bf16 [P, GI_N*KG, 512] slab layout.

    wtb[p, nt*64 + ko, nl] = weight[nt*512 + nl, ko*128 + p].
    Two-step permute keeps the expensive 8192x8192 transpose as one
    2D pass, then a cheap blocked shuffle with contiguous 1 KB runs.
    """
    wt = np.ascontiguousarray(weight.astype(ml_dtypes.bfloat16).T)  # [k, n]
    w4 = wt.reshape(KO, P, NT, 512).transpose(1, 2, 0, 3)  # [p, nt, ko, nl]
    return np.ascontiguousarray(w4).reshape(P, GI_N * KG, 512)


def make_in_maps(x, weight, weight_scale, bias):
    x = np.ascontiguousarray(np.asarray(x, dtype=np.float32))
    weight = np.asarray(weight, dtype=np.int32)
    weight_scale = np.asarray(weight_scale, dtype=np.float32).reshape(1)
    bias = np.ascontiguousarray(np.asarray(bias, dtype=np.float32))
    wtb = _pack_weight(weight)
    in_maps = []
    for c in range(8):
        in_maps.append({
            "xb": x[c * T_LOC:(c + 1) * T_LOC],
            "wtb": wtb,
            "bb": bias,
            "ws": weight_scale,
        })
    return in_maps


def kernel(x, weight, weight_scale, bias):
    nc = _build()
    in_maps = make_in_maps(x, weight, weight_scale, bias)
    res = run_bass_kernel_spmd(nc, in_maps, list(range(8))).results

    out = np.empty((TOKENS, OUT_F), dtype=np.float32)
    for c in range(8):
        out[c * T_LOC:(c + 1) * T_LOC, :] = np.asarray(
            res[c]["ob"], dtype=np.float32)
    return out
